# revision 14
# baseline (speedup 1.0000x reference)
"""DirectedGraphConvolution Trainium2 kernel (bf16 restructure).

Per batch element b (one per NeuronCore, 8 total, data-parallel):
    Ne  = H @ W                          [n, dout]
    T1  = G @ Ne   (+ rs = G @ 1)        stream phase, per arriving G tile
    A   : [cs | T2] = G.T @ [1 1 | Ne]   post-stream, fused with
    out2 = relu(G.T @ (T1 / rs))         ... same stationary blocks
    out1 = relu(0.5*(T1 + T2))
    out3 = relu(G @ (T2 / cs))           last sweep, stationary = gt blocks

Schedule: G streams from HBM split across BOTH HWDGE queues (sync: even
tiles, scalar: odd tiles; H/W first so Ne is ready early).  Arriving f32
tiles are cast to bf16 on GPSIMD, whose only other work is memsets: the
DMA-issue instructions block on staging-slot release, so the slot-freeing
caster must never sit behind other compute (a blocked scalar-engine queue
serializes the whole stream).  Per tile the PE transposes its 16 blocks (bf16
1 cyc/row, bf16 PSUM passthrough -> cheap casts) building a persistent
G^T copy, then runs the 16-matmul T1 accumulation -- so the DMA window
is filled with work that only needs *early* tiles, and everything that
needs *all* of G (pass A) runs post-stream at full PE rate.  rs/cs fall
out of ones-columns fused into the bf16 matmul rhs (no DVE reductions).
Both G (natural, for G.T-left products) and G^T (for G-left) fit
SBUF-resident in bf16.  All matmul streams are >=256 wide (1 cyc/row).
"""

import numpy as np
import concourse.bass as bass
import concourse.mybir as mybir
import concourse.tile as tile
from concourse import bacc
from concourse.bass_utils import run_bass_kernel_spmd
from concourse.masks import make_identity

F32 = mybir.dt.float32
F32R = mybir.dt.float32r
BF16 = mybir.dt.bfloat16
RELU = mybir.ActivationFunctionType.Relu
MULT = mybir.AluOpType.mult
ADD = mybir.AluOpType.add

P = 128
B = 8
N = 2048
NO = N // P            # 16 row tiles
DIN = 256
DOUT = 256
KO = DIN // P          # 2 k tiles for H @ W
W3 = 3 * DOUT
RB = 2 + DOUT          # rhs columns: [1 1 | Ne]
HH = 1024              # half a G tile's columns


def build():
    nc = bacc.Bacc("TRN2", target_bir_lowering=False)
    G = nc.declare_dram_parameter("G", [N, N], F32, isOutput=False)
    H = nc.declare_dram_parameter("H", [N, DIN], F32, isOutput=False)
    W = nc.declare_dram_parameter("W", [DIN, DOUT], F32, isOutput=False)
    out = nc.declare_dram_parameter("out", [N, W3], F32, isOutput=True)

    G_r = G.rearrange("(o p) j -> p o j", p=P)
    H_r = H.rearrange("(o p) d -> p o d", p=P)
    W_r = W.rearrange("(o p) d -> p o d", p=P)
    out_r = out.rearrange("(o p) d -> p o d", p=P)

    with tile.TileContext(nc) as tc:
        with (
            tc.tile_pool(name="const", bufs=1) as const,
            tc.tile_pool(name="gpool", bufs=1) as gpool,
            tc.tile_pool(name="gtpool", bufs=1) as gtpool,
            tc.tile_pool(name="bufp", bufs=1) as bufp,
            tc.tile_pool(name="tpp", bufs=1) as tpp,
            tc.tile_pool(name="stg", bufs=2) as stg,
            tc.tile_pool(name="stage", bufs=1) as stage,
            tc.tile_pool(name="tmpp", bufs=2) as tmpp,
        ):
            # ---------- DMA issue: H/W first, then G interleaved ----------
            hs1 = stg.tile([P, NO // 2, DIN], F32, tag="hs1", bufs=1, name="hs1")
            nc.sync.dma_start(hs1, H_r[:, 0:NO // 2, :])
            ws = const.tile([P, KO, DOUT], F32)
            nc.scalar.dma_start(ws, W_r)
            hs2 = stg.tile([P, NO // 2, DIN], F32, tag="hs2", bufs=1, name="hs2")
            nc.scalar.dma_start(hs2, H_r[:, NO // 2:NO, :])
            # G tiles staged in f32 halves; even tiles on the sync queue,
            # odd on scalar.  Slot-release waits pace each queue; the
            # f32->bf16 casts that free slots run on the OTHER engine.
            gst = {}
            for it in range(NO):
                eng = nc.sync if it % 2 == 0 else nc.scalar
                qt = "gs" if it % 2 == 0 else "gc"
                ha = stg.tile([P, HH], F32, tag=qt + "a", name=f"g{it}a")
                eng.dma_start(ha, G_r[:, it, 0:HH])
                hb = stg.tile([P, HH], F32, tag=qt + "b", bufs=1, name=f"g{it}b")
                eng.dma_start(hb, G_r[:, it, HH:N])
                gst[it] = (ha, hb)

            # ---------- constants / persistent tiles ----------
            ident_f32 = const.tile([P, P], F32)
            make_identity(nc, ident_f32)
            ident16 = const.tile([P, P], BF16)
            nc.vector.tensor_copy(ident16, ident_f32)
            w16 = const.tile([P, KO, DOUT], BF16)
            nc.vector.tensor_copy(w16, ws)
            rs_sb = const.tile([P, NO, 1], F32)

            g16 = [gpool.tile([P, N], BF16, tag=f"g{o}", name=f"g16_{o}")
                   for o in range(NO)]
            gt = [gtpool.tile([P, N], BF16, tag=f"t{o}", name=f"gt_{o}")
                  for o in range(NO)]
            # rhs buffer per block: [1 1 | Ne]  (bf16), packed in one tile
            # so the 516B rows don't each pad to a full slot
            bufall = bufp.tile([P, NO, RB], BF16, name="bufall")
            buf = [bufall[:, o, :] for o in range(NO)]
            t1p = [tpp.tile([P, DOUT], BF16, tag=f"p{o}", name=f"t1p{o}")
                   for o in range(NO)]
            t2p = [tpp.tile([P, DOUT], BF16, tag=f"q{o}", name=f"t2p{o}")
                   for o in range(NO)]
            for o in range(NO):
                nc.gpsimd.memset(buf[o][:, 0:2], 1.0)

            # ---------- Ne = H @ W ----------
            # H blocks transposed on PE straight from the f32 staging
            # (f32r, 1.5 cyc/row), cast to bf16, matmul'd against W.
            with (
                tc.tile_pool(name="ps_ht", bufs=2, space="PSUM") as ps_ht,
                tc.tile_pool(name="ps_ne", bufs=2, space="PSUM") as ps_ne,
            ):
                for rnd in range(2):
                    hs = hs1 if rnd == 0 else hs2
                    hts = stg.tile([P, 8 * KO * P], BF16, tag="hts", bufs=1,
                                   name=f"hts{rnd}")
                    for q in range(4):  # 4 psum fills of 4 transposes
                        pq = ps_ht.tile([P, 4 * P], F32, tag="pht")
                        for u in range(4):
                            blk = q * 4 + u          # t-kt block index in hs
                            t, kt = blk // KO, blk % KO
                            nc.tensor.transpose(
                                pq[:, u * P:(u + 1) * P],
                                hs[:, t, kt * P:(kt + 1) * P],
                                ident_f32,
                            )
                        nc.vector.tensor_copy(
                            hts[:, q * 4 * P:(q + 1) * 4 * P], pq)
                    for t in range(8):
                        tglob = rnd * 8 + t
                        pne = ps_ne.tile([P, DOUT], F32, tag="pne")
                        for kt in range(KO):
                            nc.tensor.matmul(
                                pne,
                                hts[:, (t * KO + kt) * P:(t * KO + kt + 1) * P],
                                w16[:, kt, :],
                                start=(kt == 0),
                                stop=(kt == KO - 1),
                            )
                        nc.vector.tensor_copy(buf[tglob][:, 2:RB], pne)

            # ---------- stream phase: per G tile ----------
            # cast f32->bf16, transpose 16 blocks (bf16 PSUM), cast to gt,
            # then T1[it] (+rs) = gt[it].T-blocks @ [1 1 | Ne].
            with (
                tc.tile_pool(name="ps_tr", bufs=2, space="PSUM") as ps_tr,
                tc.tile_pool(name="ps_t1", bufs=2, space="PSUM") as ps_t1,
            ):
                for it in range(NO):
                    ha, hb = gst[it]
                    # gpsimd frees staging slots: it has no other work, so
                    # the DMA queues never wait behind a busy compute engine
                    nc.gpsimd.tensor_copy(g16[it][:, 0:HH], ha)
                    nc.gpsimd.tensor_copy(g16[it][:, HH:N], hb)
                    tra = ps_tr.tile([P, 8 * P], BF16, tag="tra")
                    trb = ps_tr.tile([P, 8 * P], BF16, tag="trb")
                    for jt in range(8):
                        nc.tensor.transpose(
                            tra[:, jt * P:(jt + 1) * P],
                            g16[it][:, jt * P:(jt + 1) * P],
                            ident16,
                        )
                    for jt in range(8, NO):
                        nc.tensor.transpose(
                            trb[:, (jt - 8) * P:(jt - 7) * P],
                            g16[it][:, jt * P:(jt + 1) * P],
                            ident16,
                        )
                    nc.vector.tensor_copy(gt[it][:, 0:HH], tra)
                    nc.vector.tensor_copy(gt[it][:, HH:N], trb)
                    pt1 = ps_t1.tile([P, RB], F32, tag="pt1")
                    for jt in range(NO):
                        nc.tensor.matmul(
                            pt1,
                            gt[it][:, jt * P:(jt + 1) * P],
                            buf[jt][:, 0:RB],
                            start=(jt == 0),
                            stop=(jt == NO - 1),
                        )
                    # epilogue: rs, T1' = T1/rs (bf16)
                    nc.vector.tensor_copy(rs_sb[:, it, :], pt1[:, 0:1])
                    rsinv = tmpp.tile([P, 1], F32, tag="rsi")
                    nc.vector.reciprocal(rsinv, pt1[:, 0:1])
                    nc.vector.tensor_scalar_mul(
                        t1p[it], pt1[:, 2:RB], rsinv[:, 0:1])

            # ---------- fused pass A + out2 (stationary = g16 blocks) ----
            # pa = G.T @ [1 1 | Ne] -> [cs | T2];  po2 = G.T @ T1'
            with (
                tc.tile_pool(name="ps_a", bufs=2, space="PSUM") as ps_a,
                tc.tile_pool(name="ps_o2", bufs=2, space="PSUM") as ps_o2,
            ):
                for jt in range(NO):
                    pa = ps_a.tile([P, RB], F32, tag="pa")
                    po2 = ps_o2.tile([P, DOUT], F32, tag="po2")
                    for it in range(NO):
                        nc.tensor.matmul(
                            pa,
                            g16[it][:, jt * P:(jt + 1) * P],
                            buf[it][:, 0:RB],
                            start=(it == 0),
                            stop=(it == NO - 1),
                        )
                        nc.tensor.matmul(
                            po2,
                            g16[it][:, jt * P:(jt + 1) * P],
                            t1p[it],
                            start=(it == 0),
                            stop=(it == NO - 1),
                        )
                    # epilogue: T2' = T2/cs; out1 = relu(0.5(T1 + T2));
                    # out2 = relu(po2)
                    csinv = tmpp.tile([P, 1], F32, tag="csi")
                    nc.vector.reciprocal(csinv, pa[:, 0:1])
                    nc.vector.tensor_scalar_mul(
                        t2p[jt], pa[:, 2:RB], csinv[:, 0:1])
                    o1p = tmpp.tile([P, DOUT], F32, tag="o1p", bufs=1)
                    nc.vector.scalar_tensor_tensor(
                        o1p, t1p[jt], rs_sb[:, jt, :], pa[:, 2:RB], MULT, ADD)
                    o1 = stage.tile([P, DOUT], F32, tag="o", bufs=2, name="o1")
                    nc.scalar.activation(o1, o1p, RELU, scale=0.5)
                    nc.sync.dma_start(out_r[:, jt, 0:DOUT], o1)
                    o2 = stage.tile([P, DOUT], F32, tag="o", bufs=2, name="o2")
                    nc.scalar.activation(o2, po2, RELU)
                    nc.sync.dma_start(out_r[:, jt, DOUT:2 * DOUT], o2)

            # ---------- out3 = relu(G @ T2') (stationary = gt blocks) ----
            with tc.tile_pool(name="ps_o3", bufs=3, space="PSUM") as ps_o3:
                for it in range(NO):
                    po3 = ps_o3.tile([P, DOUT], F32, tag="po3")
                    for jt in range(NO):
                        nc.tensor.matmul(
                            po3,
                            gt[it][:, jt * P:(jt + 1) * P],
                            t2p[jt],
                            start=(jt == 0),
                            stop=(jt == NO - 1),
                        )
                    o3 = stage.tile([P, DOUT], F32, tag="o", bufs=2, name="o3")
                    nc.scalar.activation(o3, po3, RELU)
                    nc.sync.dma_start(out_r[:, it, 2 * DOUT:W3], o3)

    nc.compile()
    return nc


_NC = None


def _get_nc():
    global _NC
    if _NC is None:
        _NC = build()
    return _NC


def run(inputs: dict, trace: bool = False):
    """Run on 8 cores; returns (stacked_out [B,N,W3], BassKernelResults)."""
    H, G, W = inputs["H"], inputs["G"], inputs["W"]
    H = np.ascontiguousarray(H, dtype=np.float32)
    G = np.ascontiguousarray(G, dtype=np.float32)
    W = np.ascontiguousarray(W, dtype=np.float32)
    in_maps = [
        {"G": np.ascontiguousarray(G[b]), "H": np.ascontiguousarray(H[b]), "W": W}
        for b in range(B)
    ]
    nc = _get_nc()
    res = run_bass_kernel_spmd(nc, in_maps, core_ids=list(range(B)), trace=trace)
    out = np.stack([res.results[b]["out"] for b in range(B)], axis=0)
    return out, res


def kernel(H, G, W):
    out, _ = run({"H": H, "G": G, "W": W})
    return out


# revision 18
# speedup vs baseline: 1.2966x; 1.2966x over previous
"""DirectedGraphConvolution Trainium2 kernel (bf16 restructure).

Per batch element b (one per NeuronCore, 8 total, data-parallel):
    Ne  = H @ W                          [n, dout]
    T1  = G @ Ne   (+ rs = G @ 1)        stream phase, per arriving G tile
    A   : [cs | T2] = G.T @ [1 1 | Ne]   post-stream, fused with
    out2 = relu(G.T @ (T1 / rs))         ... same stationary blocks
    out1 = relu(0.5*(T1 + T2))
    out3 = relu(G @ (T2 / cs))           last sweep, stationary = gt blocks

Schedule: G streams from HBM on the sync queue (SP engine: its DMA-issue
instructions block on staging-slot release, and SP has no compute, so
blocking is free); H/W go alone on the scalar queue so Ne is ready early
and ACT never stalls on a slot wait.  Arriving f32 tiles are cast to bf16
split DVE/ACT.  Per tile the PE transposes its 16 blocks (bf16
1 cyc/row, bf16 PSUM passthrough -> cheap casts) building a persistent
G^T copy, then runs the 16-matmul T1 accumulation -- so the DMA window
is filled with work that only needs *early* tiles, and everything that
needs *all* of G (pass A) runs post-stream at full PE rate.  rs/cs fall
out of ones-columns fused into the bf16 matmul rhs (no DVE reductions).
Both G (natural, for G.T-left products) and G^T (for G-left) fit
SBUF-resident in bf16.  All matmul streams are >=256 wide (1 cyc/row).
"""

import numpy as np
import concourse.bass as bass
import concourse.mybir as mybir
import concourse.tile as tile
from concourse import bacc
from concourse.bass_utils import run_bass_kernel_spmd
from concourse.masks import make_identity

F32 = mybir.dt.float32
F32R = mybir.dt.float32r
BF16 = mybir.dt.bfloat16
RELU = mybir.ActivationFunctionType.Relu
MULT = mybir.AluOpType.mult
ADD = mybir.AluOpType.add

P = 128
B = 8
N = 2048
NO = N // P            # 16 row tiles
DIN = 256
DOUT = 256
KO = DIN // P          # 2 k tiles for H @ W
W3 = 3 * DOUT
RB = 2 + DOUT          # rhs columns: [1 1 | Ne]
HH = 1024              # half a G tile's columns


def build():
    nc = bacc.Bacc("TRN2", target_bir_lowering=False)
    G = nc.declare_dram_parameter("G", [N, N], F32, isOutput=False)
    H = nc.declare_dram_parameter("H", [N, DIN], F32, isOutput=False)
    W = nc.declare_dram_parameter("W", [DIN, DOUT], F32, isOutput=False)
    out = nc.declare_dram_parameter("out", [N, W3], F32, isOutput=True)

    G_r = G.rearrange("(o p) j -> p o j", p=P)
    H_r = H.rearrange("(o p) d -> p o d", p=P)
    W_r = W.rearrange("(o p) d -> p o d", p=P)
    out_r = out.rearrange("(o p) d -> p o d", p=P)

    with tile.TileContext(nc) as tc:
        with (
            tc.tile_pool(name="const", bufs=1) as const,
            tc.tile_pool(name="gpool", bufs=1) as gpool,
            tc.tile_pool(name="gtpool", bufs=1) as gtpool,
            tc.tile_pool(name="bufp", bufs=1) as bufp,
            tc.tile_pool(name="tpp", bufs=1) as tpp,
            tc.tile_pool(name="stg", bufs=2) as stg,
            tc.tile_pool(name="stage", bufs=1) as stage,
            tc.tile_pool(name="tmpp", bufs=2) as tmpp,
        ):
            # ---------- DMA issue ----------
            # H + W alone on the scalar queue (dedicated tiles, so the ACT
            # engine's issue instructions never block on slot waits); ALL of
            # G on the sync queue -- its staging-slot waits land on SP,
            # which has no compute, so blocking there is free.
            hs1 = stg.tile([P, NO // 2, DIN], F32, tag="hs1", bufs=1, name="hs1")
            nc.scalar.dma_start(hs1, H_r[:, 0:NO // 2, :])
            hs2 = stg.tile([P, NO // 2, DIN], F32, tag="hs2", bufs=1, name="hs2")
            nc.scalar.dma_start(hs2, H_r[:, NO // 2:NO, :])
            ws = const.tile([P, KO, DOUT], F32)
            nc.scalar.dma_start(ws, W_r)
            gst = {}
            for it in range(NO):
                ha = stg.tile([P, HH], F32, tag="gsa", bufs=3, name=f"g{it}a")
                nc.sync.dma_start(ha, G_r[:, it, 0:HH])
                hb = stg.tile([P, HH], F32, tag="gsb", bufs=3, name=f"g{it}b")
                nc.sync.dma_start(hb, G_r[:, it, HH:N])
                gst[it] = (ha, hb)

            # ---------- constants / persistent tiles ----------
            ident_f32 = const.tile([P, P], F32)
            make_identity(nc, ident_f32)
            ident16 = const.tile([P, P], BF16)
            nc.vector.tensor_copy(ident16, ident_f32)
            w16 = const.tile([P, KO, DOUT], BF16)
            nc.vector.tensor_copy(w16, ws)
            rs_sb = const.tile([P, NO, 1], F32)

            g16 = [gpool.tile([P, N], BF16, tag=f"g{o}", name=f"g16_{o}")
                   for o in range(NO)]
            gt = [gtpool.tile([P, N], BF16, tag=f"t{o}", name=f"gt_{o}")
                  for o in range(NO)]
            # rhs buffer per block: [1 1 | Ne]  (bf16), packed in one tile
            # so the 516B rows don't each pad to a full slot
            bufall = bufp.tile([P, NO, RB], BF16, name="bufall")
            buf = [bufall[:, o, :] for o in range(NO)]
            t1p = [tpp.tile([P, DOUT], BF16, tag=f"p{o}", name=f"t1p{o}")
                   for o in range(NO)]
            t2p = [tpp.tile([P, DOUT], BF16, tag=f"q{o}", name=f"t2p{o}")
                   for o in range(NO)]
            for o in range(NO):
                nc.gpsimd.memset(buf[o][:, 0:2], 1.0)

            # ---------- Ne = H @ W ----------
            # H blocks transposed on PE straight from the f32 staging
            # (f32r, 1.5 cyc/row), cast to bf16, matmul'd against W.
            with (
                tc.tile_pool(name="ps_ht", bufs=2, space="PSUM") as ps_ht,
                tc.tile_pool(name="ps_ne", bufs=2, space="PSUM") as ps_ne,
            ):
                for rnd in range(2):
                    hs = hs1 if rnd == 0 else hs2
                    hts = stg.tile([P, 8 * KO * P], BF16, tag="hts", bufs=1,
                                   name=f"hts{rnd}")
                    for q in range(4):  # 4 psum fills of 4 transposes
                        pq = ps_ht.tile([P, 4 * P], F32, tag="pht")
                        for u in range(4):
                            blk = q * 4 + u          # t-kt block index in hs
                            t, kt = blk // KO, blk % KO
                            nc.tensor.transpose(
                                pq[:, u * P:(u + 1) * P],
                                hs[:, t, kt * P:(kt + 1) * P],
                                ident_f32,
                            )
                        nc.vector.tensor_copy(
                            hts[:, q * 4 * P:(q + 1) * 4 * P], pq)
                    for t in range(8):
                        tglob = rnd * 8 + t
                        pne = ps_ne.tile([P, DOUT], F32, tag="pne")
                        for kt in range(KO):
                            nc.tensor.matmul(
                                pne,
                                hts[:, (t * KO + kt) * P:(t * KO + kt + 1) * P],
                                w16[:, kt, :],
                                start=(kt == 0),
                                stop=(kt == KO - 1),
                            )
                        nc.vector.tensor_copy(buf[tglob][:, 2:RB], pne)

            # ---------- stream phase: per G tile ----------
            # cast f32->bf16, transpose 16 blocks (bf16 PSUM), cast to gt,
            # then T1[it] (+rs) = gt[it].T-blocks @ [1 1 | Ne].
            with (
                tc.tile_pool(name="ps_tr", bufs=2, space="PSUM") as ps_tr,
                tc.tile_pool(name="ps_t1", bufs=2, space="PSUM") as ps_t1,
            ):
                for it in range(NO):
                    ha, hb = gst[it]
                    # slot-freeing casts split DVE/ACT; ACT has no blocking
                    # DMA issues (G is on sync), so slots free promptly
                    nc.vector.tensor_copy(g16[it][:, 0:HH], ha)
                    nc.scalar.copy(g16[it][:, HH:N], hb)
                    tra = ps_tr.tile([P, 8 * P], BF16, tag="tra")
                    trb = ps_tr.tile([P, 8 * P], BF16, tag="trb")
                    for jt in range(8):
                        nc.tensor.transpose(
                            tra[:, jt * P:(jt + 1) * P],
                            g16[it][:, jt * P:(jt + 1) * P],
                            ident16,
                        )
                    for jt in range(8, NO):
                        nc.tensor.transpose(
                            trb[:, (jt - 8) * P:(jt - 7) * P],
                            g16[it][:, jt * P:(jt + 1) * P],
                            ident16,
                        )
                    nc.vector.tensor_copy(gt[it][:, 0:HH], tra)
                    nc.scalar.copy(gt[it][:, HH:N], trb)
                    pt1 = ps_t1.tile([P, RB], F32, tag="pt1")
                    for jt in range(NO):
                        nc.tensor.matmul(
                            pt1,
                            gt[it][:, jt * P:(jt + 1) * P],
                            buf[jt][:, 0:RB],
                            start=(jt == 0),
                            stop=(jt == NO - 1),
                        )
                    # epilogue: rs, T1' = T1/rs (bf16)
                    nc.vector.tensor_copy(rs_sb[:, it, :], pt1[:, 0:1])
                    rsinv = tmpp.tile([P, 1], F32, tag="rsi")
                    nc.vector.reciprocal(rsinv, pt1[:, 0:1])
                    nc.vector.tensor_scalar_mul(
                        t1p[it], pt1[:, 2:RB], rsinv[:, 0:1])

            # ---------- fused pass A + out2 (stationary = g16 blocks) ----
            # pa = G.T @ [1 1 | Ne] -> [cs | T2];  po2 = G.T @ T1'
            with (
                tc.tile_pool(name="ps_a", bufs=2, space="PSUM") as ps_a,
                tc.tile_pool(name="ps_o2", bufs=2, space="PSUM") as ps_o2,
            ):
                for jt in range(NO):
                    pa = ps_a.tile([P, RB], F32, tag="pa")
                    po2 = ps_o2.tile([P, DOUT], F32, tag="po2")
                    for it in range(NO):
                        nc.tensor.matmul(
                            pa,
                            g16[it][:, jt * P:(jt + 1) * P],
                            buf[it][:, 0:RB],
                            start=(it == 0),
                            stop=(it == NO - 1),
                        )
                        nc.tensor.matmul(
                            po2,
                            g16[it][:, jt * P:(jt + 1) * P],
                            t1p[it],
                            start=(it == 0),
                            stop=(it == NO - 1),
                        )
                    # epilogue: T2' = T2/cs; out1 = relu(0.5(T1 + T2));
                    # out2 = relu(po2)
                    csinv = tmpp.tile([P, 1], F32, tag="csi")
                    nc.vector.reciprocal(csinv, pa[:, 0:1])
                    nc.vector.tensor_scalar_mul(
                        t2p[jt], pa[:, 2:RB], csinv[:, 0:1])
                    o1p = tmpp.tile([P, DOUT], F32, tag="o1p", bufs=1)
                    nc.vector.scalar_tensor_tensor(
                        o1p, t1p[jt], rs_sb[:, jt, :], pa[:, 2:RB], MULT, ADD)
                    o1 = stage.tile([P, DOUT], F32, tag="o", bufs=2, name="o1")
                    nc.scalar.activation(o1, o1p, RELU, scale=0.5)
                    nc.sync.dma_start(out_r[:, jt, 0:DOUT], o1)
                    o2 = stage.tile([P, DOUT], F32, tag="o", bufs=2, name="o2")
                    nc.scalar.activation(o2, po2, RELU)
                    nc.sync.dma_start(out_r[:, jt, DOUT:2 * DOUT], o2)

            # ---------- out3 = relu(G @ T2') (stationary = gt blocks) ----
            with tc.tile_pool(name="ps_o3", bufs=3, space="PSUM") as ps_o3:
                for it in range(NO):
                    po3 = ps_o3.tile([P, DOUT], F32, tag="po3")
                    for jt in range(NO):
                        nc.tensor.matmul(
                            po3,
                            gt[it][:, jt * P:(jt + 1) * P],
                            t2p[jt],
                            start=(jt == 0),
                            stop=(jt == NO - 1),
                        )
                    o3 = stage.tile([P, DOUT], F32, tag="o", bufs=2, name="o3")
                    nc.scalar.activation(o3, po3, RELU)
                    nc.sync.dma_start(out_r[:, it, 2 * DOUT:W3], o3)

    nc.compile()
    return nc


_NC = None


def _get_nc():
    global _NC
    if _NC is None:
        _NC = build()
    return _NC


def run(inputs: dict, trace: bool = False):
    """Run on 8 cores; returns (stacked_out [B,N,W3], BassKernelResults)."""
    H, G, W = inputs["H"], inputs["G"], inputs["W"]
    H = np.ascontiguousarray(H, dtype=np.float32)
    G = np.ascontiguousarray(G, dtype=np.float32)
    W = np.ascontiguousarray(W, dtype=np.float32)
    in_maps = [
        {"G": np.ascontiguousarray(G[b]), "H": np.ascontiguousarray(H[b]), "W": W}
        for b in range(B)
    ]
    nc = _get_nc()
    res = run_bass_kernel_spmd(nc, in_maps, core_ids=list(range(B)), trace=trace)
    out = np.stack([res.results[b]["out"] for b in range(B)], axis=0)
    return out, res


def kernel(H, G, W):
    out, _ = run({"H": H, "G": G, "W": W})
    return out
